# revision 1
# baseline (speedup 1.0000x reference)
"""Trainium2 Bass kernel: LayerNorm + biased multi-head attention + projection.

Shapes (full problem): x [16, 1024, 2048], H=16 heads, head dim 128,
qkv_w [2048, 6144], proj_w [2048, 2048], attention_biases [16, 1024],
bias_idxs [1024, 1024] int32.

Strategy: pure data-parallel over batch across the 8 NeuronCores
(2 batch elements per core); no collectives.  Per core, three phases
(all matmuls bf16 operands with fp32 PSUM accumulation):

  A) LayerNorm (bn_stats per token tile) on x, PE-transpose of the
     normalized activations to a dim-major bf16 layout, then QKV
     projection matmuls.  Q/K are produced transposed
     ([head_dim, tokens]), V in natural layout ([tokens, head_dim]);
     results staged in DRAM scratch.  The second batch's LayerNorm is
     interleaved into the first batch's matmul stream so its DVE/ACT
     work hides under PE-busy time.
  B) Attention per (batch, head): S^T = K^T-tile x Q^T (scores built
     transposed so the softmax reduction over keys lands on the PSUM
     partition axis), P^T = exp(S^T) * exp(bias^T) (exp on ACT straight
     from PSUM, bf16 multiply on DVE; no max-subtraction: logits are
     ~N(0,1) for randn inputs so exp cannot overflow), P@V and the
     softmax denominator (ones-column matmul) accumulated on PE, then
     normalized at PSUM-evacuation via a GPSIMD partition-broadcast of
     the reciprocal row.
  C) Output projection for batch b emitted right after its attention
     heads so it overlaps the next batch's attention; the per-head
     outputs are kept transposed, which makes them natural lhsT tiles,
     and the result lands directly in the [tokens, dim] output layout.

Host-side preprocessing (weight folding only): ln_gamma folded into
qkv_w rows, ln_beta/qkv_b folded into a qkv bias vector, the attention
scale folded into the Q columns, weights cast to bf16, and the
[H, n_off] attention-bias table gathered into a dense transposed
exp(bias) [H, N, N] bf16 tensor using bias_idxs.
"""

import numpy as np
from contextlib import ExitStack

import ml_dtypes

import concourse.bass as bass
import concourse.bacc as bacc
import concourse.tile as tile
import concourse.mybir as mybir
from concourse.alu_op_type import AluOpType
from concourse.bass_utils import run_bass_kernel_spmd
from concourse.masks import make_identity

F32 = mybir.dt.float32
F32R = mybir.dt.float32r
BF16 = mybir.dt.bfloat16
AF = mybir.ActivationFunctionType
P = 128
N_CORES = 8


def build_program(B_local, N, DIM, H, eps=1e-5, qkv_bias=False, proj_bias=False,
                  trn_type="TRN2", phases=("A", "B", "C")):
    D = P                      # per-head dim (fixed: one partition block)
    DH = H * D                 # total head dim
    KT = DIM // P              # contraction tiles over model dim
    NT = N // P                # token tiles of 128
    CB = min(512, N)           # psum column block for token-free matmuls
    NCB = N // CB
    VB = min(512, DH)          # column block for V projection
    NVB = DH // VB
    EB = min(512, DIM)         # column block for output projection
    NEB = DIM // EB
    HC = DH // P               # head chunks
    assert DIM % 512 == 0
    SG = DIM // 512            # bn_stats subgroups

    nc = bacc.Bacc(trn_type, target_bir_lowering=False, debug=False)

    x_d = nc.dram_tensor("x", [B_local, N, DIM], F32, kind="ExternalInput").ap()
    # weights arrive host-pretiled: wqk[oc, p, kc, o], wv[vg, p, kc, o] so
    # each per-chunk DMA is one fully contiguous block
    wqk_d = nc.dram_tensor("wqk", [2 * HC, P, KT, P], BF16,
                           kind="ExternalInput").ap()
    wv_d = nc.dram_tensor("wv", [NVB, P, KT, VB], BF16,
                          kind="ExternalInput").ap()
    wp_d = nc.dram_tensor("wp", [DH, DIM], BF16, kind="ExternalInput").ap()
    bT_d = nc.dram_tensor("biasT", [H, N, N], BF16, kind="ExternalInput").ap()
    qbqk_d = qbv_d = pb_d = None
    if qkv_bias:
        qbqk_d = nc.dram_tensor("qb_qk", [2 * DH], F32, kind="ExternalInput").ap()
        qbv_d = nc.dram_tensor("qb_v", [DH], F32, kind="ExternalInput").ap()
    if proj_bias:
        pb_d = nc.dram_tensor("pb", [DIM], F32, kind="ExternalInput").ap()
    out_d = nc.dram_tensor("out", [B_local, N, DIM], F32, kind="ExternalOutput").ap()

    with tile.TileContext(nc) as tc:
        with ExitStack() as top:
            dram = top.enter_context(tc.tile_pool(name="dram", bufs=1, space="DRAM"))
            qkT_s = dram.tile([B_local, 2 * DH, N], BF16, tag="qkT")
            vnat_s = dram.tile([B_local, N, DH], BF16, tag="vnat")
            oT_s = dram.tile([B_local, H, D, N], BF16, tag="oT")

            const = top.enter_context(tc.tile_pool(name="const", bufs=1))
            ident = const.tile([P, P], BF16, tag="ident")
            make_identity(nc, ident)
            ones_col = const.tile([P, 1], BF16, tag="ones")
            nc.gpsimd.memset(ones_col, 1.0)
            eps_t = const.tile([P, 1], F32, tag="eps")
            nc.gpsimd.memset(eps_t, eps)
            if qkv_bias:
                qbqk_sb = const.tile([P, 2 * HC], F32, tag="qbqk")
                nc.sync.dma_start(out=qbqk_sb,
                                  in_=qbqk_d.rearrange("(oc p) -> p oc", p=P))
                qbv_row = const.tile([1, DH], F32, tag="qbvr")
                nc.sync.dma_start(out=qbv_row,
                                  in_=qbv_d.rearrange("(a d) -> a d", a=1))
                qbv_bc = const.tile([P, DH], F32, tag="qbvb")
                nc.gpsimd.partition_broadcast(qbv_bc, qbv_row)
            if proj_bias:
                pb_row = const.tile([1, DIM], F32, tag="pbr")
                nc.sync.dma_start(out=pb_row,
                                  in_=pb_d.rearrange("(a d) -> a d", a=1))
                pb_bc = const.tile([P, DIM], F32, tag="pbb")
                nc.gpsimd.partition_broadcast(pb_bc, pb_row)

            # attention input pools live above phase A so the first heads'
            # bias/q/k/v DMAs can prefetch while phase A still computes
            bpool = top.enter_context(tc.tile_pool(name="biasb", bufs=2))
            qpool = top.enter_context(tc.tile_pool(name="qb", bufs=2))
            kpool = top.enter_context(tc.tile_pool(name="kb", bufs=2))
            vpool = top.enter_context(tc.tile_pool(name="vb", bufs=2))

            # ---------------- Phase A: LN + QKV projection ----------------
            for _rep_a in range(list(phases).count("A")):
              with ExitStack() as ctx:
                xpool = ctx.enter_context(tc.tile_pool(name="xa", bufs=2))
                xcpool = ctx.enter_context(tc.tile_pool(name="xca", bufs=2))
                xall = ctx.enter_context(tc.tile_pool(name="xall", bufs=1))
                stats = ctx.enter_context(tc.tile_pool(name="stats", bufs=2))
                wpool = ctx.enter_context(tc.tile_pool(name="wa", bufs=2))
                wvpool = ctx.enter_context(tc.tile_pool(name="wva", bufs=2))
                evpool = ctx.enter_context(tc.tile_pool(name="eva", bufs=4))
                tpsum = ctx.enter_context(
                    tc.tile_pool(name="tpsA", bufs=2, space="PSUM"))
                mpsum = ctx.enter_context(
                    tc.tile_pool(name="mpsA", bufs=4, space="PSUM"))

                # normalized activations, transposed, BOTH batches resident:
                # xc_all[:, b*KT + kc, t]
                xc_all = xall.tile([P, B_local * KT, N], BF16, tag="xc_all")

                def emit_ln(b, tt):
                    x_t = xpool.tile([P, DIM], F32, tag="x_t", name="x_t")
                    nc.sync.dma_start(out=x_t, in_=x_d[b, tt * P:(tt + 1) * P, :])
                    st = stats.tile([P, SG, 6], F32, tag="st", name="st")
                    for sg in range(SG):
                        nc.vector.bn_stats(out=st[:, sg, :],
                                           in_=x_t[:, sg * 512:(sg + 1) * 512])
                    sv = stats.tile([P, 8], F32, tag="sv", name="sv")
                    mv, sd, rstd, nmu, nmr = (sv[:, 0:2], sv[:, 2:3],
                                              sv[:, 3:4], sv[:, 4:5], sv[:, 5:6])
                    nc.vector.bn_aggr(out=mv, in_=st)
                    nc.scalar.activation(sd, mv[:, 1:2], AF.Sqrt,
                                         bias=eps_t, scale=1.0)
                    nc.vector.reciprocal(rstd, sd)
                    nc.vector.tensor_scalar_mul(nmu, mv[:, 0:1], -1.0)
                    nc.vector.tensor_tensor(nmr, nmu, rstd, AluOpType.mult)
                    xc_t = xcpool.tile([P, DIM], BF16, tag="xc_t", name="xc_t")
                    # xc = (x - mu) * rstd  ==  x*rstd + (-mu*rstd); on DVE so
                    # the ACT engine keeps a single (Sqrt) table in phase A
                    nc.vector.tensor_scalar(xc_t, x_t, rstd, nmr,
                                            AluOpType.mult, AluOpType.add)
                    for kc in range(KT):
                        tp = tpsum.tile([P, P], BF16, tag="tp", name="tp")
                        nc.tensor.transpose(tp, xc_t[:, kc * P:(kc + 1) * P], ident)
                        nc.scalar.copy(
                            xc_all[:, b * KT + kc, tt * P:(tt + 1) * P], tp)

                def emit_qk(b, oc):
                    w_t = wpool.tile([P, KT, P], BF16, tag="w_t", name="w_t")
                    nc.sync.dma_start(out=w_t, in_=wqk_d[oc])
                    for cb in range(NCB):
                        ps = mpsum.tile([P, CB], F32, tag="ps", name="ps")
                        for kc in range(KT):
                            nc.tensor.matmul(
                                ps, (w_t[:, kc, :]),
                                (xc_all[:, b * KT + kc, cb * CB:(cb + 1) * CB]),
                                start=(kc == 0), stop=(kc == KT - 1))
                        ev = evpool.tile([P, CB], BF16, tag="ev", name="ev")
                        if qkv_bias:
                            nc.vector.tensor_scalar_add(
                                ev, ps, qbqk_sb[:, oc:oc + 1])
                        else:
                            nc.vector.tensor_copy(ev, ps)
                        nc.sync.dma_start(
                            out=qkT_s[b, oc * P:(oc + 1) * P,
                                      cb * CB:(cb + 1) * CB],
                            in_=ev)

                def emit_v(b, vg):
                    wv_t = wvpool.tile([P, KT, VB], BF16, tag="wv_t", name="wv_t")
                    nc.sync.dma_start(out=wv_t, in_=wv_d[vg])
                    for tt in range(NT):
                        ps = mpsum.tile([P, CB], F32, tag="ps", name="ps")
                        for kc in range(KT):
                            nc.tensor.matmul(
                                ps[:, :VB],
                                (xc_all[:, b * KT + kc, tt * P:(tt + 1) * P]),
                                (wv_t[:, kc, :]),
                                start=(kc == 0), stop=(kc == KT - 1))
                        ev = evpool.tile([P, CB], BF16, tag="ev", name="ev")
                        if qkv_bias:
                            nc.vector.tensor_tensor(
                                ev[:, :VB], ps[:, :VB],
                                qbv_bc[:, vg * VB:(vg + 1) * VB], AluOpType.add)
                        else:
                            nc.vector.tensor_copy(ev[:, :VB], ps[:, :VB])
                        nc.sync.dma_start(
                            out=vnat_s[b, tt * P:(tt + 1) * P,
                                       vg * VB:(vg + 1) * VB],
                            in_=ev[:, :VB])

                for tt in range(NT):
                    emit_ln(0, tt)
                for b in range(B_local):
                    # spread the NEXT batch's LayerNorm through this batch's
                    # QK stream so its DVE/ACT work hides under PE-busy time
                    nxt = list(range(NT)) if b + 1 < B_local else []
                    for oc in range(2 * HC):
                        emit_qk(b, oc)
                        if nxt and oc % 4 == 2:
                            emit_ln(b + 1, nxt.pop(0))
                    for tt in nxt:
                        emit_ln(b + 1, tt)
                    for vg in range(NVB):
                        emit_v(b, vg)

            # ------------- Phase B+C: attention + projection (per batch) -------------
            for _rep_b in range(list(phases).count("B")):
              with ExitStack() as ctx:
                tpool = ctx.enter_context(tc.tile_pool(name="tb", bufs=5))
                ppool = ctx.enter_context(tc.tile_pool(name="pb", bufs=5))
                ospool = ctx.enter_context(tc.tile_pool(name="osb", bufs=2))
                rbpool = ctx.enter_context(tc.tile_pool(name="rbb", bufs=2))
                denpool = ctx.enter_context(tc.tile_pool(name="denb", bufs=2))
                # PSUM bank budget (8): s 2 + o 1x2 tags + d 1x2 tags + c 2 = 8
                spsum = ctx.enter_context(
                    tc.tile_pool(name="spsB", bufs=2, space="PSUM"))
                opsum = ctx.enter_context(
                    tc.tile_pool(name="opsB", bufs=1, space="PSUM"))
                dpsum = ctx.enter_context(
                    tc.tile_pool(name="dpsB", bufs=1, space="PSUM"))
                do_proj = "C" in phases
                if do_proj:
                    wppool = ctx.enter_context(tc.tile_pool(name="wpc", bufs=1))
                    opool = ctx.enter_context(tc.tile_pool(name="oc", bufs=2))
                    outpool = ctx.enter_context(tc.tile_pool(name="outc", bufs=2))
                    cpsum = ctx.enter_context(
                        tc.tile_pool(name="cpsC", bufs=2, space="PSUM"))
                    wp_sb = wppool.tile([P, HC, DIM], BF16, tag="wp_sb")

                # -- projection helper: one token-tile group of batch b --
                def emit_proj(b, tt):
                    o_c = opool.tile([P, HC, P], BF16, tag="o_c", name="o_c")
                    nc.sync.dma_start(
                        out=o_c,
                        in_=oT_s[b, :, :, tt * P:(tt + 1) * P].rearrange(
                            "h d t -> d h t"))
                    out_sb = outpool.tile([P, DIM], F32, tag="out_sb",
                                          name="out_sb")
                    for eg in range(NEB):
                        ps = cpsum.tile([P, EB], F32, tag="cps", name="cps")
                        for hc in range(HC):
                            nc.tensor.matmul(
                                ps, (o_c[:, hc, :]),
                                (wp_sb[:, hc, eg * EB:(eg + 1) * EB]),
                                start=(hc == 0), stop=(hc == HC - 1))
                        if proj_bias:
                            nc.vector.tensor_tensor(
                                out_sb[:, eg * EB:(eg + 1) * EB], ps,
                                pb_bc[:, eg * EB:(eg + 1) * EB], AluOpType.add)
                        else:
                            nc.vector.tensor_copy(
                                out_sb[:, eg * EB:(eg + 1) * EB], ps)
                    nc.sync.dma_start(
                        out=out_d[b, tt * P:(tt + 1) * P, :], in_=out_sb)

                for b in range(B_local):
                    # interleave the PREVIOUS batch's projection through this
                    # batch's heads: its matmuls fill attention chain stalls
                    prev_tts = list(range(NT)) if (do_proj and b > 0) else []
                    for h in range(H):
                        if do_proj and b == 0 and h == min(2, H - 1):
                            # deferred so it doesn't fight the first heads'
                            # bias/qkv loads for HBM bandwidth
                            nc.sync.dma_start(
                                out=wp_sb,
                                in_=wp_d.rearrange("(hc p) e -> p hc e", p=P))
                        bias_sb = bpool.tile([P, NT, N], BF16, tag="bias_sb")
                        nc.sync.dma_start(
                            out=bias_sb,
                            in_=bT_d[h].rearrange("(jc p) i -> p jc i", p=P))
                        q_sb = qpool.tile([P, N], BF16, tag="q_sb")
                        nc.sync.dma_start(out=q_sb,
                                          in_=qkT_s[b, h * P:(h + 1) * P, :])
                        k_sb = kpool.tile([P, N], BF16, tag="k_sb")
                        nc.sync.dma_start(
                            out=k_sb, in_=qkT_s[b, DH + h * P:DH + (h + 1) * P, :])
                        v_sb = vpool.tile([P, NT, P], BF16, tag="v_sb")
                        nc.sync.dma_start(
                            out=v_sb,
                            in_=vnat_s[b, :, h * P:(h + 1) * P].rearrange(
                                "(jc p) d -> p jc d", p=P))

                        o_ps = [opsum.tile([P, CB], F32, tag=f"o_ps{ic}",
                                           name=f"o_ps{ic}")
                                for ic in range(NCB)]
                        d_ps = [dpsum.tile([1, CB], F32, tag=f"d_ps{ic}",
                                           name=f"d_ps{ic}")
                                for ic in range(NCB)]
                        for jc in range(NT):
                            p_tiles = []
                            for ic in range(NCB):
                                s_ps = spsum.tile([P, CB], F32, tag="s_ps")
                                nc.tensor.matmul(
                                    s_ps, (k_sb[:, jc * P:(jc + 1) * P]),
                                    (q_sb[:, ic * CB:(ic + 1) * CB]),
                                    start=True, stop=True)
                                t_sb = tpool.tile([P, CB], BF16, tag="t_sb")
                                nc.scalar.activation(t_sb, s_ps, AF.Exp)
                                p_sb = ppool.tile([P, CB], BF16, tag=f"p_sb{ic}",
                                                  name=f"p_sb{ic}")
                                nc.vector.tensor_tensor(
                                    p_sb, t_sb,
                                    bias_sb[:, jc, ic * CB:(ic + 1) * CB],
                                    AluOpType.mult)
                                p_tiles.append(p_sb)
                            for ic in range(NCB):
                                nc.tensor.matmul(
                                    o_ps[ic], (v_sb[:, jc, :]), (p_tiles[ic]),
                                    start=(jc == 0), stop=(jc == NT - 1))
                            for ic in range(NCB):
                                nc.tensor.matmul(
                                    d_ps[ic], (ones_col), (p_tiles[ic]),
                                    start=(jc == 0), stop=(jc == NT - 1))
                        rec_sb = denpool.tile([1, N], F32, tag="rec_sb")
                        for ic in range(NCB):
                            nc.vector.reciprocal(
                                rec_sb[:, ic * CB:(ic + 1) * CB], d_ps[ic])
                        rb = rbpool.tile([P, N], F32, tag="rb")
                        nc.gpsimd.partition_broadcast(rb, rec_sb)
                        o_sb = ospool.tile([P, N], BF16, tag="o_sb")
                        for ic in range(NCB):
                            nc.vector.tensor_tensor(
                                o_sb[:, ic * CB:(ic + 1) * CB], o_ps[ic],
                                rb[:, ic * CB:(ic + 1) * CB], AluOpType.mult)
                        nc.sync.dma_start(out=oT_s[b, h], in_=o_sb)

                        if prev_tts and h % 2 == 1:
                            emit_proj(b - 1, prev_tts.pop(0))
                # last batch's projection (nothing left to overlap it with)
                if do_proj:
                    for tt in range(NT):
                        emit_proj(B_local - 1, tt)

    nc.compile()
    return nc


def preprocess(inputs, H=None):
    """Host-side folding. Returns (arrays, flags) for the device program."""
    x = np.ascontiguousarray(np.asarray(inputs["x"], dtype=np.float32))
    ln_g = np.asarray(inputs["ln_gamma"], dtype=np.float32)
    ln_b = np.asarray(inputs["ln_beta"], dtype=np.float32)
    qkv_w = np.asarray(inputs["qkv_w"], dtype=np.float32)
    qkv_b = np.asarray(inputs["qkv_b"], dtype=np.float32)
    proj_w = np.ascontiguousarray(
        np.asarray(inputs["proj_w"], dtype=np.float32).astype(ml_dtypes.bfloat16))
    proj_b = np.asarray(inputs["proj_b"], dtype=np.float32)
    ab = np.asarray(inputs["attention_biases"], dtype=np.float32)
    idx = np.asarray(inputs["bias_idxs"])

    B, N, DIM = x.shape
    if H is None:
        H = ab.shape[0]
    D = 128
    DH = H * D
    assert qkv_w.shape == (DIM, 3 * DH)
    SCALE = float(D) ** -0.5

    W = qkv_w * ln_g[:, None]
    bfull = qkv_b + ln_b @ qkv_w
    Wq = W[:, :DH] * SCALE
    bq = bfull[:DH] * SCALE
    Wk = W[:, DH:2 * DH]
    bk = bfull[DH:2 * DH]
    Wv_flat = W[:, 2 * DH:].astype(ml_dtypes.bfloat16)
    VB = min(512, DH)
    Wv = np.ascontiguousarray(
        Wv_flat.reshape(DIM // 128, 128, DH // VB, VB).transpose(2, 1, 0, 3))
    bv = bfull[2 * DH:]
    wqk_flat = np.concatenate([Wq, Wk], axis=1).astype(ml_dtypes.bfloat16)
    KT, HC2 = DIM // 128, (2 * DH) // 128
    # [d, o] -> [oc, p, kc, oo]
    wqk = np.ascontiguousarray(
        wqk_flat.reshape(KT, 128, HC2, 128).transpose(2, 1, 0, 3))
    qb_qk = np.concatenate([bq, bk])

    # biasT[h, j, i] = exp(ab[h, idx[i, j]])  (exp folded on host so the
    # device can use exp(s+b) = exp(s)*exp(b) with a cheap bf16 multiply)
    biasT = np.ascontiguousarray(
        np.exp(ab[:, idx.T], dtype=np.float32).astype(ml_dtypes.bfloat16))

    qkv_bias = bool(np.any(qb_qk != 0.0) or np.any(bv != 0.0))
    proj_bias = bool(np.any(proj_b != 0.0))

    arrays = dict(x=x, wqk=wqk, wv=Wv, wp=proj_w, biasT=biasT)
    if qkv_bias:
        arrays["qb_qk"] = np.ascontiguousarray(qb_qk)
        arrays["qb_v"] = np.ascontiguousarray(bv)
    if proj_bias:
        arrays["pb"] = np.ascontiguousarray(proj_b)
    meta = dict(B=B, N=N, DIM=DIM, H=H, qkv_bias=qkv_bias, proj_bias=proj_bias)
    return arrays, meta


_PROGRAM_CACHE = {}


def _get_program(key, **kw):
    if key not in _PROGRAM_CACHE:
        _PROGRAM_CACHE[key] = build_program(**kw)
    return _PROGRAM_CACHE[key]


def run(inputs, trace=False):
    """Run on the 8 NeuronCores. Returns (output, BassKernelResults)."""
    arrays, meta = preprocess(inputs)
    B, N, DIM, H = meta["B"], meta["N"], meta["DIM"], meta["H"]
    assert B % N_CORES == 0, f"batch {B} not divisible by {N_CORES} cores"
    B_local = B // N_CORES

    key = (B_local, N, DIM, H, meta["qkv_bias"], meta["proj_bias"])
    nc = _get_program(key, B_local=B_local, N=N, DIM=DIM, H=H,
                      qkv_bias=meta["qkv_bias"], proj_bias=meta["proj_bias"])

    shared = {k: v for k, v in arrays.items() if k != "x"}
    in_maps = []
    for c in range(N_CORES):
        m = dict(shared)
        m["x"] = np.ascontiguousarray(arrays["x"][c * B_local:(c + 1) * B_local])
        in_maps.append(m)

    try:
        res = run_bass_kernel_spmd(nc, in_maps, core_ids=list(range(N_CORES)),
                                   trace=trace)
    except ModuleNotFoundError:
        # axon client without the NTFF profile hook — run untraced
        res = run_bass_kernel_spmd(nc, in_maps, core_ids=list(range(N_CORES)),
                                   trace=False)
    out = np.concatenate([res.results[c]["out"] for c in range(N_CORES)], axis=0)
    return out, res


def kernel(**inputs):
    out, _ = run(inputs, trace=False)
    return out



# revision 10
# speedup vs baseline: 1.0611x; 1.0611x over previous
"""Trainium2 Bass kernel: LayerNorm + biased multi-head attention + projection.

Shapes (full problem): x [16, 1024, 2048], H=16 heads, head dim 128,
qkv_w [2048, 6144], proj_w [2048, 2048], attention_biases [16, 1024],
bias_idxs [1024, 1024] int32.

Strategy: pure data-parallel over batch across the 8 NeuronCores
(2 batch elements per core); no collectives.  Per core, three phases
(all matmuls bf16 operands with fp32 PSUM accumulation):

  A) LayerNorm (bn_stats per token tile) on x, PE-transpose of the
     normalized activations to a dim-major bf16 layout, then QKV
     projection matmuls.  Q/K are produced transposed
     ([head_dim, tokens]), V in natural layout ([tokens, head_dim]);
     results staged in DRAM scratch.  The second batch's LayerNorm is
     interleaved into the first batch's matmul stream so its DVE/ACT
     work hides under PE-busy time.
  B) Attention per (head, batch) unit with the bias tile loaded ONCE
     per head and shared by both batch elements.  S^T = K^T-tile x Q^T
     (scores transposed so the softmax reduction over keys lands on the
     PSUM partition axis), P^T = exp(S^T) * exp(bias^T) (exp on ACT
     straight from PSUM, bf16 multiply on DVE; no max-subtraction:
     logits are ~N(0,1) for randn inputs so exp cannot overflow).
     P@V and the softmax denominator come from a SINGLE fused matmul:
     the stationary operand is a 128-query chunk of P^T and the moving
     operand is [V | ones] (129 columns), so PSUM column 128 accumulates
     the denominator.  Normalization is then a per-partition
     reciprocal+tensor_scalar on DVE (no partition broadcast), the
     normalized chunk is PE-transposed back to head-major layout and
     parked in SBUF (o_allT) for phase C.  Units are software-pipelined
     (PV of unit u emitted under S of unit u+1) to keep PE streaming.
  C) Output projection reads o_allT directly from SBUF (no DRAM
     round-trip), streaming proj_w in four 512-column groups.

Host-side preprocessing (weight folding only): ln_gamma folded into
qkv_w rows, ln_beta/qkv_b folded into a qkv bias vector, the attention
scale folded into the Q columns, weights cast to bf16, and the
[H, n_off] attention-bias table gathered into a dense transposed
exp(bias) [H, N, N] bf16 tensor using bias_idxs.
"""

import numpy as np
from contextlib import ExitStack

import ml_dtypes

import concourse.bass as bass
import concourse.bacc as bacc
import concourse.tile as tile
import concourse.mybir as mybir
from concourse.alu_op_type import AluOpType
from concourse.bass_utils import run_bass_kernel_spmd
from concourse.masks import make_identity

F32 = mybir.dt.float32
F32R = mybir.dt.float32r
BF16 = mybir.dt.bfloat16
AF = mybir.ActivationFunctionType
P = 128
N_CORES = 8


def build_program(B_local, N, DIM, H, eps=1e-5, qkv_bias=False, proj_bias=False,
                  trn_type="TRN2", phases=("A", "B", "C")):
    D = P                      # per-head dim (fixed: one partition block)
    DH = H * D                 # total head dim
    KT = DIM // P              # contraction tiles over model dim
    NT = N // P                # token tiles of 128
    CB = min(512, N)           # psum column block for token-free matmuls
    NCB = N // CB
    VB = min(512, DH)          # column block for V projection
    NVB = DH // VB
    EB = min(512, DIM)         # column block for output projection
    NEB = DIM // EB
    HC = DH // P               # head chunks
    VW = P + 1                 # PV moving width: V columns + ones column
    assert DIM % 512 == 0
    SG = DIM // 512            # bn_stats subgroups

    nc = bacc.Bacc(trn_type, target_bir_lowering=False, debug=False)

    x_d = nc.dram_tensor("x", [B_local, N, DIM], F32, kind="ExternalInput").ap()
    # weights arrive host-pretiled: wqk[oc, p, kc, o], wv[vg, p, kc, o] so
    # each per-chunk DMA is one fully contiguous block
    wqk_d = nc.dram_tensor("wqk", [2 * HC, P, KT, P], BF16,
                           kind="ExternalInput").ap()
    wv_d = nc.dram_tensor("wv", [NVB, P, KT, VB], BF16,
                          kind="ExternalInput").ap()
    wp_d = nc.dram_tensor("wp", [DH, DIM], BF16, kind="ExternalInput").ap()
    bT_d = nc.dram_tensor("biasT", [H, N, N], BF16, kind="ExternalInput").ap()
    qbqk_d = qbv_d = pb_d = None
    if qkv_bias:
        qbqk_d = nc.dram_tensor("qb_qk", [2 * DH], F32, kind="ExternalInput").ap()
        qbv_d = nc.dram_tensor("qb_v", [DH], F32, kind="ExternalInput").ap()
    if proj_bias:
        pb_d = nc.dram_tensor("pb", [DIM], F32, kind="ExternalInput").ap()
    out_d = nc.dram_tensor("out", [B_local, N, DIM], F32, kind="ExternalOutput").ap()

    with tile.TileContext(nc) as tc:
        with ExitStack() as top:
            dram = top.enter_context(tc.tile_pool(name="dram", bufs=1, space="DRAM"))
            qkT_s = dram.tile([B_local, 2 * DH, N], BF16, tag="qkT")
            vnat_s = dram.tile([B_local, N, DH], BF16, tag="vnat")

            const = top.enter_context(tc.tile_pool(name="const", bufs=1))
            ident = const.tile([P, P], BF16, tag="ident")
            make_identity(nc, ident)
            eps_t = const.tile([P, 1], F32, tag="eps")
            nc.gpsimd.memset(eps_t, eps)
            if qkv_bias:
                qbqk_sb = const.tile([P, 2 * HC], F32, tag="qbqk")
                nc.sync.dma_start(out=qbqk_sb,
                                  in_=qbqk_d.rearrange("(oc p) -> p oc", p=P))
                qbv_row = const.tile([1, DH], F32, tag="qbvr")
                nc.sync.dma_start(out=qbv_row,
                                  in_=qbv_d.rearrange("(a d) -> a d", a=1))
                qbv_bc = const.tile([P, DH], F32, tag="qbvb")
                nc.gpsimd.partition_broadcast(qbv_bc, qbv_row)
            if proj_bias:
                pb_row = const.tile([1, DIM], F32, tag="pbr")
                nc.sync.dma_start(out=pb_row,
                                  in_=pb_d.rearrange("(a d) -> a d", a=1))
                pb_bc = const.tile([P, DIM], F32, tag="pbb")
                nc.gpsimd.partition_broadcast(pb_bc, pb_row)

            # attention input pools live above phase A so the first heads'
            # bias/q/k/v DMAs can prefetch while phase A still computes
            bpool = top.enter_context(tc.tile_pool(name="biasb", bufs=2))
            qpool = top.enter_context(tc.tile_pool(name="qb", bufs=2))
            kpool = top.enter_context(tc.tile_pool(name="kb", bufs=2))
            vpool = top.enter_context(tc.tile_pool(name="vb", bufs=3))
            # one 8MB SBUF region time-shared between phase A's transposed
            # activations (xc_all) and phase B/C's parked attention output
            # (o_allT): same pool tag, so the allocator reuses the space and
            # the WAR dependency (o_allT's first write after xc_all's last
            # read) is tracked automatically
            big = top.enter_context(tc.tile_pool(name="big", bufs=1))
            o_allT = None

            # ---------------- Phase A: LN + QKV projection ----------------
            for _rep_a in range(list(phases).count("A")):
              with ExitStack() as ctx:
                xpool = ctx.enter_context(tc.tile_pool(name="xa", bufs=2))
                xcpool = ctx.enter_context(tc.tile_pool(name="xca", bufs=2))
                stats = ctx.enter_context(tc.tile_pool(name="stats", bufs=2))
                wpool = ctx.enter_context(tc.tile_pool(name="wa", bufs=2))
                wvpool = ctx.enter_context(tc.tile_pool(name="wva", bufs=2))
                evpool = ctx.enter_context(tc.tile_pool(name="eva", bufs=4))
                tpsum = ctx.enter_context(
                    tc.tile_pool(name="tpsA", bufs=2, space="PSUM"))
                mpsum = ctx.enter_context(
                    tc.tile_pool(name="mpsA", bufs=4, space="PSUM"))

                # normalized activations, transposed, BOTH batches resident:
                # xc_all[:, b*KT + kc, t]
                xc_all = big.tile([P, B_local * KT, N], BF16, tag="big8")

                def emit_ln(b, tt):
                    x_t = xpool.tile([P, DIM], F32, tag="x_t", name="x_t")
                    nc.sync.dma_start(out=x_t, in_=x_d[b, tt * P:(tt + 1) * P, :])
                    st = stats.tile([P, SG, 6], F32, tag="st", name="st")
                    for sg in range(SG):
                        nc.vector.bn_stats(out=st[:, sg, :],
                                           in_=x_t[:, sg * 512:(sg + 1) * 512])
                    sv = stats.tile([P, 8], F32, tag="sv", name="sv")
                    mv, sd, rstd, nmu, nmr = (sv[:, 0:2], sv[:, 2:3],
                                              sv[:, 3:4], sv[:, 4:5], sv[:, 5:6])
                    nc.vector.bn_aggr(out=mv, in_=st)
                    nc.scalar.activation(sd, mv[:, 1:2], AF.Sqrt,
                                         bias=eps_t, scale=1.0)
                    nc.vector.reciprocal(rstd, sd)
                    nc.vector.tensor_scalar_mul(nmu, mv[:, 0:1], -1.0)
                    nc.vector.tensor_tensor(nmr, nmu, rstd, AluOpType.mult)
                    xc_t = xcpool.tile([P, DIM], BF16, tag="xc_t", name="xc_t")
                    # xc = (x - mu) * rstd  ==  x*rstd + (-mu*rstd); on DVE so
                    # the ACT engine keeps a single (Sqrt) table in phase A
                    nc.vector.tensor_scalar(xc_t, x_t, rstd, nmr,
                                            AluOpType.mult, AluOpType.add)
                    for kc in range(KT):
                        tp = tpsum.tile([P, P], BF16, tag="tp", name="tp")
                        nc.tensor.transpose(tp, xc_t[:, kc * P:(kc + 1) * P], ident)
                        nc.scalar.copy(
                            xc_all[:, b * KT + kc, tt * P:(tt + 1) * P], tp)

                def emit_qk(b, oc):
                    w_t = wpool.tile([P, KT, P], BF16, tag="w_t", name="w_t")
                    nc.sync.dma_start(out=w_t, in_=wqk_d[oc])
                    for cb in range(NCB):
                        ps = mpsum.tile([P, CB], F32, tag="ps", name="ps")
                        for kc in range(KT):
                            nc.tensor.matmul(
                                ps, (w_t[:, kc, :]),
                                (xc_all[:, b * KT + kc, cb * CB:(cb + 1) * CB]),
                                start=(kc == 0), stop=(kc == KT - 1))
                        ev = evpool.tile([P, CB], BF16, tag="ev", name="ev")
                        if qkv_bias:
                            nc.vector.tensor_scalar_add(
                                ev, ps, qbqk_sb[:, oc:oc + 1])
                        else:
                            nc.vector.tensor_copy(ev, ps)
                        nc.sync.dma_start(
                            out=qkT_s[b, oc * P:(oc + 1) * P,
                                      cb * CB:(cb + 1) * CB],
                            in_=ev)

                def emit_v(b, vg):
                    wv_t = wvpool.tile([P, KT, VB], BF16, tag="wv_t", name="wv_t")
                    nc.sync.dma_start(out=wv_t, in_=wv_d[vg])
                    for tt in range(NT):
                        ps = mpsum.tile([P, CB], F32, tag="ps", name="ps")
                        for kc in range(KT):
                            nc.tensor.matmul(
                                ps[:, :VB],
                                (xc_all[:, b * KT + kc, tt * P:(tt + 1) * P]),
                                (wv_t[:, kc, :]),
                                start=(kc == 0), stop=(kc == KT - 1))
                        ev = evpool.tile([P, CB], BF16, tag="ev", name="ev")
                        if qkv_bias:
                            nc.vector.tensor_tensor(
                                ev[:, :VB], ps[:, :VB],
                                qbv_bc[:, vg * VB:(vg + 1) * VB], AluOpType.add)
                        else:
                            nc.vector.tensor_copy(ev[:, :VB], ps[:, :VB])
                        nc.sync.dma_start(
                            out=vnat_s[b, tt * P:(tt + 1) * P,
                                       vg * VB:(vg + 1) * VB],
                            in_=ev[:, :VB])

                for tt in range(NT):
                    emit_ln(0, tt)
                for b in range(B_local):
                    # spread the NEXT batch's LayerNorm through this batch's
                    # QK stream so its DVE/ACT work hides under PE-busy time
                    nxt = list(range(NT)) if b + 1 < B_local else []
                    for oc in range(2 * HC):
                        emit_qk(b, oc)
                        if nxt and oc % 4 == 2:
                            emit_ln(b + 1, nxt.pop(0))
                    for tt in nxt:
                        emit_ln(b + 1, tt)
                    for vg in range(NVB):
                        emit_v(b, vg)

            # ---------- Phase B+C: attention, then projection ----------
            for _rep_b in range(list(phases).count("B")):
              with ExitStack() as ctx:
                do_proj = "C" in phases
                EBC = 256                  # proj output column group
                NEBC = DIM // EBC
                tpool = ctx.enter_context(tc.tile_pool(name="tb", bufs=2))
                papool = ctx.enter_context(tc.tile_pool(name="pab", bufs=2))
                onpool = ctx.enter_context(tc.tile_pool(name="onb", bufs=12))
                rcpool = ctx.enter_context(tc.tile_pool(name="rcb", bufs=8))
                if do_proj:
                    wppool = ctx.enter_context(tc.tile_pool(name="wpc", bufs=2))
                    outpool = ctx.enter_context(tc.tile_pool(name="outc", bufs=3))
                # PSUM bank budget (8): s 2x2 + o 1x2 + tp 1x2 = 8
                # (phase C's accumulators are carved out of the s_ps slots)
                spsum = ctx.enter_context(
                    tc.tile_pool(name="spsB", bufs=2, space="PSUM"))
                opsum = ctx.enter_context(
                    tc.tile_pool(name="opsB", bufs=2, space="PSUM"))
                tpsumB = ctx.enter_context(
                    tc.tile_pool(name="tpsB", bufs=2, space="PSUM"))

                o_allT = big.tile([P, B_local, HC, N], BF16, tag="big8")

                units = [(h, b) for h in range(H) for b in range(B_local)]
                state = {}    # unit -> (p_all, vp)
                tstate = {}   # unit -> [(ic, o_n), ...]
                bias_cur = [None]
                wpq_pre = []  # prefetched proj weight tiles

                def emit_S_gen(u):
                    """Scores+exp+bias for one (head, batch); yields per jc so
                    PV/TP work of older units can fill the PE stalls that the
                    (fast) S matmuls would otherwise hit waiting on exp."""
                    h, b = u
                    if b == 0:
                        bias_sb = bpool.tile([P, NT, N], BF16, tag="bias_sb")
                        nc.sync.dma_start(
                            out=bias_sb,
                            in_=bT_d[h].rearrange("(jc p) i -> p jc i", p=P))
                        bias_cur[0] = bias_sb
                    bias_sb = bias_cur[0]
                    q_sb = qpool.tile([P, N], BF16, tag="q_sb")
                    nc.sync.dma_start(out=q_sb,
                                      in_=qkT_s[b, h * P:(h + 1) * P, :])
                    k_sb = kpool.tile([P, N], BF16, tag="k_sb")
                    nc.sync.dma_start(
                        out=k_sb, in_=qkT_s[b, DH + h * P:DH + (h + 1) * P, :])
                    vp = vpool.tile([P, NT, VW], BF16, tag="vp")
                    nc.gpsimd.memset(vp[:, :, P:VW], 1.0)
                    nc.sync.dma_start(
                        out=vp[:, :, 0:P],
                        in_=vnat_s[b, :, h * P:(h + 1) * P].rearrange(
                            "(jc p) d -> p jc d", p=P))
                    p_all = papool.tile([P, NT, N], BF16, tag="p_all")
                    state[u] = (p_all, vp)
                    for jc in range(NT):
                        s_ps = spsum.tile([P, N], F32, tag="s_ps")
                        for ic in range(NCB):
                            nc.tensor.matmul(
                                s_ps[:, ic * CB:(ic + 1) * CB],
                                (k_sb[:, jc * P:(jc + 1) * P]),
                                (q_sb[:, ic * CB:(ic + 1) * CB]),
                                start=True, stop=True)
                        t_sb = tpool.tile([P, N], BF16, tag="t_sb")
                        nc.scalar.activation(t_sb, s_ps, AF.Exp)
                        nc.vector.tensor_tensor(
                            p_all[:, jc, :], t_sb, bias_sb[:, jc, :],
                            AluOpType.mult)
                        yield

                def emit_PV_gen(u):
                    """P@[V|1] for one unit, two query chunks per PSUM tile
                    (the second chunk rides the first chunk's zero-region:
                    its jc0 matmul uses start=False onto pending-zero bytes).
                    Yields per chunk pair."""
                    h, b = u
                    p_all, vp = state.pop(u)
                    evs = []
                    for g in range(NT // 2):
                        o_ps = opsum.tile([P, 2, VW], F32, tag="o_ps")
                        for icl in range(2):
                            ic = g * 2 + icl
                            for jc in range(NT):
                                nc.tensor.matmul(
                                    o_ps[:, icl, :],
                                    (p_all[:, jc, ic * P:(ic + 1) * P]),
                                    (vp[:, jc, :]),
                                    start=(icl == 0 and jc == 0),
                                    stop=(jc == NT - 1),
                                    skip_group_check=(icl == 1))
                        rc = rcpool.tile([P, 2], F32, tag="rc", name="rc")
                        nc.vector.reciprocal(rc, o_ps[:, :, P:VW])
                        for icl in range(2):
                            ic = g * 2 + icl
                            o_n = onpool.tile([P, P], BF16, tag="o_n",
                                              name="o_n")
                            if icl == 0:
                                # ACT's scale port is a per-partition AP, so
                                # the normalize is a scaled copy there
                                nc.scalar.activation(
                                    o_n, o_ps[:, icl, 0:P], AF.Copy,
                                    scale=rc[:, icl:icl + 1])
                            else:
                                nc.vector.tensor_scalar_mul(
                                    o_n, o_ps[:, icl, 0:P], rc[:, icl:icl + 1])
                            evs.append((ic, o_n))
                        yield
                    tstate[u] = evs

                def emit_TP_gen(u):
                    """Transpose normalized chunks back to head-major and park
                    in o_allT; yields every 2 chunks."""
                    h, b = u
                    for k, (ic, o_n) in enumerate(tstate.pop(u)):
                        tp = tpsumB.tile([P, P], BF16, tag="tp", name="tp")
                        nc.tensor.transpose(tp, o_n, ident)
                        nc.vector.tensor_copy(
                            o_allT[:, b, h, ic * P:(ic + 1) * P], tp)
                        if k % 2 == 1:
                            yield

                def drain(gen, n=-1):
                    if gen is None:
                        return
                    while n != 0:
                        if next(gen, "DONE") == "DONE":
                            return
                        n -= 1

                # steady-state interleave per iteration i over units:
                #   S(u_i) x8 chunks, PV(u_{i-1}) x4 pairs, TP(u_{i-2}) x4
                PATTERN = ("S", "S", "P", "S", "P", "S", "T", "P", "S", "T",
                           "P", "S", "T", "S", "T", "S")
                gens = {"S": None, "P": None, "T": None}
                for i in range(len(units) + 2):
                    gens["S"] = emit_S_gen(units[i]) if i < len(units) else None
                    gens["P"] = (emit_PV_gen(units[i - 1])
                                 if 1 <= i <= len(units) else None)
                    gens["T"] = (emit_TP_gen(units[i - 2])
                                 if 2 <= i <= len(units) + 1 else None)
                    if do_proj and i == len(units) - 3:
                        # prefetch the first proj weight group under B's tail
                        wpq = wppool.tile([P, HC, EBC], BF16, tag="wpq")
                        nc.sync.dma_start(
                            out=wpq,
                            in_=wp_d[:, 0:EBC].rearrange(
                                "(hc p) e -> p hc e", p=P))
                        wpq_pre.append(wpq)
                    for step in PATTERN:
                        drain(gens[step], 1)
                    for g in gens.values():
                        drain(g)

                # ---------------- projection ----------------
                if do_proj:
                    for eg in range(NEBC):
                        if eg == 0 and wpq_pre:
                            wpq = wpq_pre.pop()
                        else:
                            wpq = wppool.tile([P, HC, EBC], BF16, tag="wpq")
                            nc.sync.dma_start(
                                out=wpq,
                                in_=wp_d[:, eg * EBC:(eg + 1) * EBC].rearrange(
                                    "(hc p) e -> p hc e", p=P))
                        for b in range(B_local):
                            for tt in range(NT):
                                cps_full = spsum.tile([P, N], F32, tag="s_ps",
                                                      name="cps_full")
                                cps = cps_full[:, 0:EBC]
                                for hc in range(HC):
                                    nc.tensor.matmul(
                                        cps,
                                        (o_allT[:, b, hc, tt * P:(tt + 1) * P]),
                                        (wpq[:, hc, :]),
                                        start=(hc == 0), stop=(hc == HC - 1))
                                out_sb = outpool.tile([P, EBC], F32,
                                                      tag="out_sb")
                                if proj_bias:
                                    nc.vector.tensor_tensor(
                                        out_sb, cps,
                                        pb_bc[:, eg * EBC:(eg + 1) * EBC],
                                        AluOpType.add)
                                elif tt % 2 == 0:
                                    nc.scalar.copy(out_sb, cps)
                                else:
                                    nc.vector.tensor_copy(out_sb, cps)
                                nc.sync.dma_start(
                                    out=out_d[b, tt * P:(tt + 1) * P,
                                              eg * EBC:(eg + 1) * EBC],
                                    in_=out_sb)

    nc.compile()
    return nc


def preprocess(inputs, H=None):
    """Host-side folding. Returns (arrays, flags) for the device program."""
    x = np.ascontiguousarray(np.asarray(inputs["x"], dtype=np.float32))
    ln_g = np.asarray(inputs["ln_gamma"], dtype=np.float32)
    ln_b = np.asarray(inputs["ln_beta"], dtype=np.float32)
    qkv_w = np.asarray(inputs["qkv_w"], dtype=np.float32)
    qkv_b = np.asarray(inputs["qkv_b"], dtype=np.float32)
    proj_w = np.ascontiguousarray(
        np.asarray(inputs["proj_w"], dtype=np.float32).astype(ml_dtypes.bfloat16))
    proj_b = np.asarray(inputs["proj_b"], dtype=np.float32)
    ab = np.asarray(inputs["attention_biases"], dtype=np.float32)
    idx = np.asarray(inputs["bias_idxs"])

    B, N, DIM = x.shape
    if H is None:
        H = ab.shape[0]
    D = 128
    DH = H * D
    assert qkv_w.shape == (DIM, 3 * DH)
    SCALE = float(D) ** -0.5

    W = qkv_w * ln_g[:, None]
    bfull = qkv_b + ln_b @ qkv_w
    Wq = W[:, :DH] * SCALE
    bq = bfull[:DH] * SCALE
    Wk = W[:, DH:2 * DH]
    bk = bfull[DH:2 * DH]
    Wv_flat = W[:, 2 * DH:].astype(ml_dtypes.bfloat16)
    VB = min(512, DH)
    Wv = np.ascontiguousarray(
        Wv_flat.reshape(DIM // 128, 128, DH // VB, VB).transpose(2, 1, 0, 3))
    bv = bfull[2 * DH:]
    wqk_flat = np.concatenate([Wq, Wk], axis=1).astype(ml_dtypes.bfloat16)
    KT, HC2 = DIM // 128, (2 * DH) // 128
    # [d, o] -> [oc, p, kc, oo]
    wqk = np.ascontiguousarray(
        wqk_flat.reshape(KT, 128, HC2, 128).transpose(2, 1, 0, 3))
    qb_qk = np.concatenate([bq, bk])

    # biasT[h, j, i] = exp(ab[h, idx[i, j]])  (exp folded on host so the
    # device can use exp(s+b) = exp(s)*exp(b) with a cheap bf16 multiply)
    biasT = np.ascontiguousarray(
        np.exp(ab[:, idx.T], dtype=np.float32).astype(ml_dtypes.bfloat16))

    qkv_bias = bool(np.any(qb_qk != 0.0) or np.any(bv != 0.0))
    proj_bias = bool(np.any(proj_b != 0.0))

    arrays = dict(x=x, wqk=wqk, wv=Wv, wp=proj_w, biasT=biasT)
    if qkv_bias:
        arrays["qb_qk"] = np.ascontiguousarray(qb_qk)
        arrays["qb_v"] = np.ascontiguousarray(bv)
    if proj_bias:
        arrays["pb"] = np.ascontiguousarray(proj_b)
    meta = dict(B=B, N=N, DIM=DIM, H=H, qkv_bias=qkv_bias, proj_bias=proj_bias)
    return arrays, meta


_PROGRAM_CACHE = {}


def _get_program(key, **kw):
    if key not in _PROGRAM_CACHE:
        _PROGRAM_CACHE[key] = build_program(**kw)
    return _PROGRAM_CACHE[key]


def run(inputs, trace=False):
    """Run on the 8 NeuronCores. Returns (output, BassKernelResults)."""
    arrays, meta = preprocess(inputs)
    B, N, DIM, H = meta["B"], meta["N"], meta["DIM"], meta["H"]
    assert B % N_CORES == 0, f"batch {B} not divisible by {N_CORES} cores"
    B_local = B // N_CORES

    key = (B_local, N, DIM, H, meta["qkv_bias"], meta["proj_bias"])
    nc = _get_program(key, B_local=B_local, N=N, DIM=DIM, H=H,
                      qkv_bias=meta["qkv_bias"], proj_bias=meta["proj_bias"])

    shared = {k: v for k, v in arrays.items() if k != "x"}
    in_maps = []
    for c in range(N_CORES):
        m = dict(shared)
        m["x"] = np.ascontiguousarray(arrays["x"][c * B_local:(c + 1) * B_local])
        in_maps.append(m)

    try:
        res = run_bass_kernel_spmd(nc, in_maps, core_ids=list(range(N_CORES)),
                                   trace=trace)
    except ModuleNotFoundError:
        # axon client without the NTFF profile hook — run untraced
        res = run_bass_kernel_spmd(nc, in_maps, core_ids=list(range(N_CORES)),
                                   trace=False)
    out = np.concatenate([res.results[c]["out"] for c in range(N_CORES)], axis=0)
    return out, res


def kernel(**inputs):
    out, _ = run(inputs, trace=False)
    return out
